# revision 6
# baseline (speedup 1.0000x reference)
"""Trainium2 Bass kernel for nn_BlockFourierCirculant.

Math: y = irfft( einsum('oik,bik->bok', Wf, rfft(x.reshape(b,16,256))) )
with 4096 features = 16 blocks x 256 and a 129-bin half-spectrum.

The op is linear per batch row and factorizes into three matmul stages
(data-parallel over batch across 8 cores):

  stage 1: per input block i, forward real-DFT (256x256, matrix T1)
  stage 2: per 4-bin group, a dense 128x128 spectral mix (W2)
  stage 3: per output block o, inverse real-DFT (256x256, matrix B3)

All three stages are emitted WEIGHT-STATIONARY: the DFT / mixing matrix
is the lhsT (PE-stationary operand) and the batch dimension is always
the moving free dimension, so every intermediate lands spectrum-major
([slot, batch]) in PSUM.  With that orientation the two inter-stage
data reorganizations are pure partition GATHERS (8-partition bricks
with the batch line kept contiguous) instead of element transposes:
plain SBUF->SBUF DMAs with 1 KiB contiguous descriptors that run near
the 435 GB/s fabric rate, ~2x the throughput of the DMA xbar-transpose
path and with ~60x fewer descriptors.

Layouts per core (bc = 1024 rows, split in 2 super-chunks of NB=512):
  x    DRAM [2, 128, 32, 512]  f16   xp[sc,p,r,b] = x[sc*512+b, r*128+p]
  s1   out X[p, i, h, b]  = Xhat[i, slot h*128+p]          (SBUF f16)
  g1   xg_g[8i+u, b]      = Xhat[i, slot 8g+u]             (gather)
  s2   out Y[8o+u', g, b] = Yhat[o, slot 8g+u']            (SBUF f16)
  g2   yo_h[p, b]         = Yhat[o, slot h*128+p]          (gather)
  s3   out y[m, 2o+th, b] = y[sc*512+b, (2o+th)*128+m]     (DRAM f16)

Engines: PE 320 matmuls (N=512); ACT+DVE alternate PSUM evacuation;
SP (HWDGE) issues the 128 gather DMAs; GpSimd (SWDGE) streams the
x / y HBM traffic so its ring waits never block an evacuation engine.
"""

import numpy as np

import concourse.bacc as bacc
import concourse.bass as bass
import concourse.mybir as mybir
import concourse.tile as tile
from concourse.bass_utils import run_bass_kernel_spmd

# ---------------------------------------------------------------- constants
BATCH = 8192
FEAT = 4096
BLOCK = 256
NBLK = 16  # blocks per row (in and out)
NSLOT = 256  # real spectrum slots per block
NGRP = 32  # stage-2 groups (8 slots each)
N_CORES = 8
BC = BATCH // N_CORES  # batch rows per core (1024)
NSC = 2  # super-chunks per core
NB = BC // NSC  # 512 batch rows per super-chunk

F32 = mybir.dt.float32
F16 = mybir.dt.float16


# ------------------------------------------------------------- host matrices
def _slot_map():
    """slot s* in 0..255 per block -> (comp, bin); comp 0 = cos, 1 = sin."""
    m = [(0, 0), (0, 128)]
    for k in range(1, 128):
        m.append((0, k))
        m.append((1, k))
    return m


_SLOTS = _slot_map()


def build_t1():
    """(256 s, 256 slots) forward real-DFT, matching np.fft.rfft."""
    s = np.arange(BLOCK)
    T1 = np.zeros((BLOCK, NSLOT), dtype=np.float64)
    for j, (comp, k) in enumerate(_SLOTS):
        ang = 2.0 * np.pi * k * s / BLOCK
        T1[:, j] = np.cos(ang) if comp == 0 else -np.sin(ang)
    return T1


def build_b3():
    """(256 slots, 256 t) inverse real-DFT, matching np.fft.irfft."""
    t = np.arange(BLOCK)
    B3 = np.zeros((NSLOT, BLOCK), dtype=np.float64)
    for j, (comp, k) in enumerate(_SLOTS):
        w = 1.0 if k in (0, 128) else 2.0
        ang = 2.0 * np.pi * k * t / BLOCK
        B3[j] = (w * np.cos(ang) if comp == 0 else -w * np.sin(ang)) / BLOCK
    return B3


def build_w2(W_real, W_imag):
    """(32, 128, 128) per-group mixing matrices.

    Group g covers slots 8g..8g+7.  Row 16u+i (u-major!) reads
    Xhat[i, slot 8g+u]; col 8o+u' writes Yhat[o, slot 8g+u'].
    The u-major row order matches the gather-DMA descriptor order
    (src walks (u partition, i slab); dst walks partitions 0..127).
    """
    Wr = np.asarray(W_real, dtype=np.float64)
    Wi = np.asarray(W_imag, dtype=np.float64)
    W2 = np.zeros((NGRP, 128, 128), dtype=np.float64)
    for g in range(NGRP):
        for u in range(8):
            comp_u, k_u = _SLOTS[8 * g + u]
            for up in range(8):
                comp_up, k_up = _SLOTS[8 * g + up]
                if k_u != k_up:
                    continue
                if comp_u == 0 and comp_up == 0:
                    coef = Wr[:, :, k_u]  # (o, i)
                elif comp_u == 1 and comp_up == 0:
                    coef = -Wi[:, :, k_u]
                elif comp_u == 0 and comp_up == 1:
                    coef = Wi[:, :, k_u]
                else:
                    coef = Wr[:, :, k_u]
                W2[g, 16 * u : 16 * u + 16, up::8] = coef.T  # [i, o]
    return W2


def pack_t1(T1):
    """(128, 2, 2, 128) fp16: [p, kc, h, m] = T1[kc*128+p, h*128+m]."""
    return np.ascontiguousarray(
        T1.reshape(2, 128, 2, 128).transpose(1, 0, 2, 3)
    ).astype(np.float16)


def pack_b3(B3):
    """(128, 2, 2, 128) fp16: [p, h, th, m] = B3[slot(h,p), th*128+m].

    The gather-2 DMA places Yhat[o, slot 128h+8gp+u'] on partition
    p = 16u'+gp of yo_h, so B3's rows are permuted to match.
    """
    p = np.arange(128)
    slot_in_half = 8 * (p % 16) + (p // 16)
    out = np.zeros((128, 2, 2, 128), dtype=np.float64)
    for h in (0, 1):
        out[:, h, :, :] = B3[128 * h + slot_in_half].reshape(128, 2, 128)
    return np.ascontiguousarray(out).astype(np.float16)


def pack_w2(W2):
    """(128, 32, 128) fp16: [r, g, c] = W2[g, r, c]."""
    return np.ascontiguousarray(W2.transpose(1, 0, 2)).astype(np.float16)


# ------------------------------------------------------------- device kernel
def emit_kernel(tc, outs, ins):
    nc = tc.nc
    xp = ins["xp"]  # [NSC, 128, 32, NB] f16
    yp = outs["yp"]  # [NSC, 128, 32, NB] f16

    with (
        tc.tile_pool(name="wpool", bufs=1) as wpool,
        tc.tile_pool(name="xpool", bufs=20) as xpool,
        tc.tile_pool(name="spec", bufs=3) as spec,
        tc.tile_pool(name="gat", bufs=8) as gat,
        tc.tile_pool(name="yst", bufs=6) as ystp,
        tc.tile_pool(name="psum", bufs=8, space="PSUM") as pspool,
    ):
        t1_sb = wpool.tile([128, 2, 2, 128], F16)
        w2_sb = wpool.tile([128, NGRP, 128], F16)
        b3_sb = wpool.tile([128, 2, 2, 128], F16)
        nc.gpsimd.dma_start(out=t1_sb, in_=ins["t1"])
        nc.gpsimd.dma_start(out=w2_sb, in_=ins["w2"])
        nc.gpsimd.dma_start(out=b3_sb, in_=ins["b3"])

        ncopy = 0  # alternate evacuation copies between ACT and DVE

        def evac(dst, src):
            nonlocal ncopy
            if ncopy % 2 == 0:
                nc.scalar.copy(out=dst, in_=src)
            else:
                nc.vector.tensor_copy(out=dst, in_=src)
            ncopy += 1

        xt = {}
        Xh = {}
        Yg = {}

        def load_x(sc):
            """HBM -> SBUF, one DMA per input block (128 x 2 KiB descs)."""
            for i in range(NBLK):
                t = xpool.tile([128, 2, NB], F16, tag="x", name=f"x_{sc}_{i}")
                nc.gpsimd.dma_start(out=t, in_=xp[sc, :, 2 * i : 2 * i + 2, :])
                xt[(sc, i)] = t

        def s1(sc):
            """forward DFT: X[p, i, h, b] = sum_f T1[f, h*128+p] x[i*256+f, b]."""
            X = spec.tile([128, NBLK, 2, NB], F16, tag="spec", name=f"X_{sc}")
            Xh[sc] = X
            for h in (0, 1):
                for i in range(NBLK):
                    ps = pspool.tile(
                        [128, NB], F32, tag="ps", name=f"ps1_{sc}_{h}_{i}"
                    )
                    x_t = xt[(sc, i)]
                    nc.tensor.matmul(
                        ps, t1_sb[:, 0, h, :], x_t[:, 0, :], start=True, stop=False
                    )
                    nc.tensor.matmul(
                        ps, t1_sb[:, 1, h, :], x_t[:, 1, :], start=False, stop=True
                    )
                    evac(X[:, i, h, :], ps)
            for i in range(NBLK):
                del xt[(sc, i)]

        def s2(sc):
            """per-group spectral mix, gather-in from X."""
            X = Xh.pop(sc)
            Y = spec.tile([128, NGRP, NB], F16, tag="spec", name=f"Y_{sc}")
            Yg[sc] = Y
            for g in range(NGRP):
                gm, hg = g % 16, g // 16
                xg = gat.tile([128, NB], F16, tag="xg", name=f"xg_{sc}_{g}")
                # xg[16u+i, b] = X[8gm+u, i, hg, b]: src descriptors walk
                # (u partition, i slab), dst walks partitions 0..127.
                nc.sync.dma_start(
                    out=xg,
                    in_=X[8 * gm : 8 * gm + 8, :, hg, :],
                )
                ps = pspool.tile([128, NB], F32, tag="ps", name=f"ps2_{sc}_{g}")
                nc.tensor.matmul(ps, w2_sb[:, g, :], xg, start=True, stop=True)
                evac(Y[:, g, :], ps)

        def s3(sc):
            """inverse DFT per output block, gather-in from Y, store y."""
            Y = Yg.pop(sc)
            for o in range(NBLK):
                # yo_h[16u'+gp, b] = Y[8o+u', 16h+gp, b]
                #                  = Yhat[o, slot 128h+8gp+u']  (B3 rows match)
                yo0 = gat.tile([128, NB], F16, tag="yo", name=f"yo0_{sc}_{o}")
                nc.sync.dma_start(out=yo0, in_=Y[8 * o : 8 * o + 8, 0:16, :])
                yo1 = gat.tile([128, NB], F16, tag="yo", name=f"yo1_{sc}_{o}")
                nc.sync.dma_start(out=yo1, in_=Y[8 * o : 8 * o + 8, 16:32, :])
                yb = ystp.tile([128, 2, NB], F16, tag="yb", name=f"yb_{sc}_{o}")
                for th in (0, 1):
                    ps = pspool.tile(
                        [128, NB], F32, tag="ps", name=f"ps3_{sc}_{o}_{th}"
                    )
                    nc.tensor.matmul(
                        ps, b3_sb[:, 0, th, :], yo0, start=True, stop=False
                    )
                    nc.tensor.matmul(
                        ps, b3_sb[:, 1, th, :], yo1, start=False, stop=True
                    )
                    evac(yb[:, th, :], ps)
                nc.gpsimd.dma_start(out=yp[sc, :, 2 * o : 2 * o + 2, :], in_=yb)

        load_x(0)
        load_x(1)
        s1(0)
        s1(1)
        s2(0)
        s2(1)
        s3(0)
        s3(1)


# ------------------------------------------------------------ host interface
_CACHED = {}


def make_inputs(W_real, W_imag):
    return {
        "t1": pack_t1(build_t1()),
        "w2": pack_w2(build_w2(W_real, W_imag)),
        "b3": pack_b3(build_b3()),
    }


def _build_bass():
    if "nc" in _CACHED:
        return _CACHED["nc"]
    nc = bacc.Bacc("TRN2", target_bir_lowering=False, debug=False)
    ins = {
        "xp": nc.dram_tensor("xp", [NSC, 128, 32, NB], F16, kind="ExternalInput").ap(),
        "t1": nc.dram_tensor("t1", [128, 2, 2, 128], F16, kind="ExternalInput").ap(),
        "w2": nc.dram_tensor("w2", [128, NGRP, 128], F16, kind="ExternalInput").ap(),
        "b3": nc.dram_tensor("b3", [128, 2, 2, 128], F16, kind="ExternalInput").ap(),
    }
    outs = {
        "yp": nc.dram_tensor("yp", [NSC, 128, 32, NB], F16, kind="ExternalOutput").ap()
    }
    with tile.TileContext(nc) as tc:
        emit_kernel(tc, outs, ins)
    nc.compile()
    _CACHED["nc"] = nc
    return nc


def run_sharded(x, W_real, W_imag, trace=False):
    """Run on 8 NeuronCores; returns (y, BassKernelResults)."""
    x = np.asarray(x, dtype=np.float32)
    w = make_inputs(W_real, W_imag)

    in_maps = []
    for c in range(N_CORES):
        xc = x[c * BC : (c + 1) * BC, :]
        # xp[sc, p, r, b] = x[sc*NB+b, r*128+p]
        xp = np.ascontiguousarray(
            xc.reshape(NSC, NB, 32, 128).transpose(0, 3, 2, 1)
        ).astype(np.float16)
        in_maps.append({"xp": xp, **w})

    nc = _build_bass()
    res = run_bass_kernel_spmd(nc, in_maps, core_ids=list(range(N_CORES)), trace=trace)

    y = np.empty((BATCH, FEAT), dtype=np.float32)
    for c in range(N_CORES):
        ypc = res.results[c]["yp"]  # [NSC, 128, 32, NB] f16
        y[c * BC : (c + 1) * BC, :] = (
            ypc.transpose(0, 3, 2, 1).reshape(BC, FEAT).astype(np.float32)
        )
    return y, res


def kernel(x, W_real, W_imag):
    y, _ = run_sharded(x, W_real, W_imag, trace=False)
    return y


# revision 15
# speedup vs baseline: 1.3300x; 1.3300x over previous
"""Trainium2 Bass kernel for nn_BlockFourierCirculant.

Math: y = irfft( einsum('oik,bik->bok', Wf, rfft(x.reshape(b,16,256))) )
with 4096 features = 16 blocks x 256 and a 129-bin half-spectrum.

The op is linear per batch row and factorizes into three matmul stages
(data-parallel over batch across 8 cores):

  stage 1: per input block i, forward real-DFT (256x256, matrix T1)
  stage 2: per 4-bin group, a dense 128x128 spectral mix (W2)
  stage 3: per output block o, inverse real-DFT (256x256, matrix B3)

All three stages are emitted WEIGHT-STATIONARY: the DFT / mixing matrix
is the lhsT (PE-stationary operand) and the batch dimension is always
the moving free dimension, so every intermediate lands spectrum-major
([slot, batch]) in PSUM.  With that orientation the two inter-stage
reorganizations are pure partition GATHERS (8 source partitions x 16
slabs -> 128 destination partitions, batch line contiguous): plain
SBUF->SBUF DMAs with 2 KiB descriptors, no xbar transpose.

All gathers (and the y stores) issue from the POOL/SWDGE sequencer:
the Q7 descriptor generator runs ahead of the 16 SDMA engines, so the
engines drift across several in-flight gathers and their 8-partition
source bricks spread over many SBUF read ports.  x loads and weights
ride the SP HWDGE ring in parallel.

Layouts:
  X  partition q of half h = Xhat[., slot 128h + q]        (natural)
  gather-1 for gm: src partitions [8gm, 8gm+8) x (i, h, b) ->
      xg2[16u+i, h, b] = Xhat[i, slot 128h+8gm+u]          (W2 rows)
  Y  partition c = 8o+u'   = Yhat[o, slot 8g+u']           (W2 cols)
  gather-2 for o:  src partitions [8o, 8o+8) x (gp, h, b) ->
      yo2[16u'+gp, h, b] = Yhat[o, slot 128h+8gp+u']       (B3 rows)
"""

import numpy as np

import concourse.bacc as bacc
import concourse.bass as bass
import concourse.mybir as mybir
import concourse.tile as tile
from concourse.bass_utils import run_bass_kernel_spmd

# ---------------------------------------------------------------- constants
BATCH = 8192
FEAT = 4096
BLOCK = 256
NBLK = 16  # blocks per row (in and out)
NSLOT = 256  # real spectrum slots per block
NGRP = 32  # stage-2 groups (8 slots each)
N_CORES = 8
BC = BATCH // N_CORES  # batch rows per core (1024)
NSC = 2  # super-chunks per core
NB = BC // NSC  # 512 batch rows per super-chunk

F32 = mybir.dt.float32
F16 = mybir.dt.float16

# ------------------------------------------------------------- host matrices
def _slot_map():
    """slot s* in 0..255 per block -> (comp, bin); comp 0 = cos, 1 = sin."""
    m = [(0, 0), (0, 128)]
    for k in range(1, 128):
        m.append((0, k))
        m.append((1, k))
    return m


_SLOTS = _slot_map()


def build_t1():
    """(256 s, 256 slots) forward real-DFT, matching np.fft.rfft."""
    s = np.arange(BLOCK)
    T1 = np.zeros((BLOCK, NSLOT), dtype=np.float64)
    for j, (comp, k) in enumerate(_SLOTS):
        ang = 2.0 * np.pi * k * s / BLOCK
        T1[:, j] = np.cos(ang) if comp == 0 else -np.sin(ang)
    return T1


def build_b3():
    """(256 slots, 256 t) inverse real-DFT, matching np.fft.irfft."""
    t = np.arange(BLOCK)
    B3 = np.zeros((NSLOT, BLOCK), dtype=np.float64)
    for j, (comp, k) in enumerate(_SLOTS):
        w = 1.0 if k in (0, 128) else 2.0
        ang = 2.0 * np.pi * k * t / BLOCK
        B3[j] = (w * np.cos(ang) if comp == 0 else -w * np.sin(ang)) / BLOCK
    return B3


def build_w2(W_real, W_imag):
    """(32, 128, 128) per-group mixing matrices.

    Group g covers slots 8g..8g+7.  Row 16u+i reads Xhat[i, slot 8g+u]
    (matching gather-1's descriptor order); col 8o+u' writes
    Yhat[o, slot 8g+u'] (so gather-2's source brick for o is the
    contiguous partition range [8o, 8o+8)).
    """
    Wr = np.asarray(W_real, dtype=np.float64)
    Wi = np.asarray(W_imag, dtype=np.float64)
    W2 = np.zeros((NGRP, 128, 128), dtype=np.float64)
    for g in range(NGRP):
        for u in range(8):
            comp_u, k_u = _SLOTS[8 * g + u]
            for up in range(8):
                comp_up, k_up = _SLOTS[8 * g + up]
                if k_u != k_up:
                    continue
                if comp_u == 0 and comp_up == 0:
                    coef = Wr[:, :, k_u]  # (o, i)
                elif comp_u == 1 and comp_up == 0:
                    coef = -Wi[:, :, k_u]
                elif comp_u == 0 and comp_up == 1:
                    coef = Wi[:, :, k_u]
                else:
                    coef = Wr[:, :, k_u]
                W2[g, 16 * u : 16 * u + 16, up::8] = coef.T  # [i, o]
    return W2


def pack_t1(T1):
    """(128, 2, 2, 128) fp16: [p, kc, h, m] = T1[kc*128+p, h*128+m]."""
    return np.ascontiguousarray(
        T1.reshape(2, 128, 2, 128).transpose(1, 0, 2, 3)
    ).astype(np.float16)


def pack_b3(B3):
    """(128, 2, 2, 128) fp16: [p, h, th, m] = B3[slot(h,p), th*128+m].

    Gather-2 puts Yhat[o, slot 128h+8gp+u'] on partition p = 16u'+gp of
    yo2, so row p corresponds to slot-in-half 8*(p%16) + p//16.
    """
    p = np.arange(128)
    sih = 8 * (p % 16) + (p // 16)
    out = np.zeros((128, 2, 2, 128), dtype=np.float64)
    for h in (0, 1):
        out[:, h, :, :] = B3[128 * h + sih].reshape(128, 2, 128)
    return np.ascontiguousarray(out).astype(np.float16)


def pack_w2(W2):
    """(128, 32, 128) fp16: [r, g, c] = W2[g, r, c]."""
    return np.ascontiguousarray(W2.transpose(1, 0, 2)).astype(np.float16)


# ------------------------------------------------------------- device kernel
def emit_kernel(tc, outs, ins):
    nc = tc.nc
    xp = ins["xp"]  # [NSC, 128, 32, NB] f16
    yp = outs["yp"]  # [NSC, 128, 32, NB] f16

    with (
        tc.tile_pool(name="wpool", bufs=1) as wpool,
        tc.tile_pool(name="xpool", bufs=6) as xpool,
        tc.tile_pool(name="spec", bufs=3) as spec,
        tc.tile_pool(name="gat", bufs=5) as gat,
        tc.tile_pool(name="yst", bufs=2) as ystp,
        tc.tile_pool(name="psum", bufs=8, space="PSUM") as pspool,
    ):
        t1_sb = wpool.tile([128, 2, 2, 128], F16)
        w2_sb = wpool.tile([128, NGRP, 128], F16)
        b3_sb = wpool.tile([128, 2, 2, 128], F16)
        nc.sync.dma_start(out=t1_sb, in_=ins["t1"])
        nc.sync.dma_start(out=w2_sb, in_=ins["w2"])
        nc.sync.dma_start(out=b3_sb, in_=ins["b3"])

        ncopy = 0  # alternate evacuation copies between ACT and DVE

        def evac(dst, src):
            nonlocal ncopy
            if ncopy % 2 == 0:
                nc.scalar.copy(out=dst, in_=src)
            else:
                nc.vector.tensor_copy(out=dst, in_=src)
            ncopy += 1

        xt = {}
        Xh = {}
        Yg = {}

        def load_x(sc):
            """HBM -> SBUF in 4 quarter DMAs (1024 x 2 KiB descriptors)."""
            for q in range(4):
                t = xpool.tile([128, 8, NB], F16, tag="x", name=f"x_{sc}_{q}")
                nc.sync.dma_start(out=t, in_=xp[sc, :, 8 * q : 8 * q + 8, :])
                xt[(sc, q)] = t

        def s1(sc):
            """forward DFT: X[p, i, h, b] = Xhat[i, slot(h,p)]."""
            X = spec.tile([128, NBLK, 2, NB], F16, tag="spec", name=f"X_{sc}")
            Xh[sc] = X
            for h in (0, 1):
                for iq in range(0, NBLK, 4):
                    pss = [
                        pspool.tile(
                            [128, NB], F32, tag="ps", name=f"ps1_{sc}_{h}_{iq+j}"
                        )
                        for j in range(4)
                    ]
                    # kc-pass interleave: the 4 independent chains hide the
                    # same-bank accumulate latency of back-to-back matmuls.
                    for kc in (0, 1):
                        for j in range(4):
                            i = iq + j
                            rhs = xt[(sc, i // 4)][:, 2 * (i % 4) + kc, :]
                            nc.tensor.matmul(
                                pss[j],
                                t1_sb[:, kc, h, :],
                                rhs,
                                start=(kc == 0),
                                stop=(kc == 1),
                            )
                    for j in range(4):
                        evac(X[:, iq + j, h, :], pss[j])
            for q in range(4):
                del xt[(sc, q)]

        def s2(sc):
            """per-group spectral mix, gather-in from X."""
            X = Xh.pop(sc)
            Xr = X.rearrange("p i h b -> p i (h b)")
            Y = spec.tile([128, 16, 2, NB], F16, tag="spec", name=f"Y_{sc}")
            Yg[sc] = Y
            for gm in range(16):
                xg = gat.tile([128, 2, NB], F16, tag="xg", name=f"xg_{sc}_{gm}")
                # xg[16u+i, hg, b] = X[8gm+u, i, hg, b]
                nc.gpsimd.dma_start(out=xg, in_=Xr[8 * gm : 8 * gm + 8])
                for hg in (0, 1):
                    g = 16 * hg + gm
                    ps = pspool.tile([128, NB], F32, tag="ps", name=f"ps2_{sc}_{g}")
                    nc.tensor.matmul(
                        ps, w2_sb[:, g, :], xg[:, hg, :], start=True, stop=True
                    )
                    evac(Y[:, gm, hg, :], ps)

        def s3(sc):
            """inverse DFT per output block, gather-in from Y, store y."""
            Y = Yg.pop(sc)
            Yr = Y.rearrange("c gp h b -> c gp (h b)")
            yb = None
            for op_ in range(0, NBLK, 2):
                if op_ % 4 == 0:
                    yb = ystp.tile(
                        [128, 8, NB], F16, tag="yb", name=f"yb_{sc}_{op_ // 4}"
                    )
                yo = {}
                for o in (op_, op_ + 1):
                    t = gat.tile([128, 2, NB], F16, tag="yo", name=f"yo_{sc}_{o}")
                    # yo[16u'+gp, h, b] = Y[8o+u', gp, h, b]
                    nc.gpsimd.dma_start(out=t, in_=Yr[8 * o : 8 * o + 8])
                    yo[o] = t
                pss = {}
                for o in (op_, op_ + 1):
                    for th in (0, 1):
                        pss[(o, th)] = pspool.tile(
                            [128, NB], F32, tag="ps", name=f"ps3_{sc}_{o}_{th}"
                        )
                # h-pass interleave over 4 independent chains
                for h in (0, 1):
                    for th in (0, 1):
                        for o in (op_, op_ + 1):
                            nc.tensor.matmul(
                                pss[(o, th)],
                                b3_sb[:, h, th, :],
                                yo[o][:, h, :],
                                start=(h == 0),
                                stop=(h == 1),
                            )
                for o in (op_, op_ + 1):
                    for th in (0, 1):
                        evac(yb[:, 2 * (o % 4) + th, :], pss[(o, th)])
                if op_ % 4 == 2:
                    oq = op_ - 2
                    nc.gpsimd.dma_start(
                        out=yp[sc, :, 2 * oq : 2 * oq + 8, :], in_=yb
                    )

        load_x(0)
        load_x(1)
        s1(0)
        s1(1)
        s2(0)
        s2(1)
        s3(0)
        s3(1)


# ------------------------------------------------------------ host interface
_CACHED = {}


def make_inputs(W_real, W_imag):
    return {
        "t1": pack_t1(build_t1()),
        "w2": pack_w2(build_w2(W_real, W_imag)),
        "b3": pack_b3(build_b3()),
    }


def _build_bass():
    if "nc" in _CACHED:
        return _CACHED["nc"]
    nc = bacc.Bacc("TRN2", target_bir_lowering=False, debug=False)
    ins = {
        "xp": nc.dram_tensor("xp", [NSC, 128, 32, NB], F16, kind="ExternalInput").ap(),
        "t1": nc.dram_tensor("t1", [128, 2, 2, 128], F16, kind="ExternalInput").ap(),
        "w2": nc.dram_tensor("w2", [128, NGRP, 128], F16, kind="ExternalInput").ap(),
        "b3": nc.dram_tensor("b3", [128, 2, 2, 128], F16, kind="ExternalInput").ap(),
    }
    outs = {
        "yp": nc.dram_tensor("yp", [NSC, 128, 32, NB], F16, kind="ExternalOutput").ap()
    }
    with tile.TileContext(nc) as tc:
        emit_kernel(tc, outs, ins)
    nc.compile()
    _CACHED["nc"] = nc
    return nc


def run_sharded(x, W_real, W_imag, trace=False):
    """Run on 8 NeuronCores; returns (y, BassKernelResults)."""
    x = np.asarray(x, dtype=np.float32)
    w = make_inputs(W_real, W_imag)

    in_maps = []
    for c in range(N_CORES):
        xc = x[c * BC : (c + 1) * BC, :]
        # xp[sc, p, r, b] = x[sc*NB+b, r*128+p]
        xp = np.ascontiguousarray(
            xc.reshape(NSC, NB, 32, 128).transpose(0, 3, 2, 1)
        ).astype(np.float16)
        in_maps.append({"xp": xp, **w})

    nc = _build_bass()
    res = run_bass_kernel_spmd(nc, in_maps, core_ids=list(range(N_CORES)), trace=trace)

    y = np.empty((BATCH, FEAT), dtype=np.float32)
    for c in range(N_CORES):
        ypc = res.results[c]["yp"]  # [NSC, 128, 32, NB] f16
        y[c * BC : (c + 1) * BC, :] = (
            ypc.transpose(0, 3, 2, 1).reshape(BC, FEAT).astype(np.float32)
        )
    return y, res


def kernel(x, W_real, W_imag):
    y, _ = run_sharded(x, W_real, W_imag, trace=False)
    return y


# revision 22
# speedup vs baseline: 1.5088x; 1.1344x over previous
"""Trainium2 Bass kernel for nn_BlockFourierCirculant.

Math: y = irfft( einsum('oik,bik->bok', Wf, rfft(x.reshape(b,16,256))) )
with 4096 features = 16 blocks x 256 and a 129-bin half-spectrum.

The op is linear per batch row and factorizes into three matmul stages
(data-parallel over batch across 8 cores):

  stage 1: per input block i, forward real-DFT (256x256, matrix T1)
  stage 2: per frequency bin, a 32x32 real "complex multiply + block mix",
           bundled 4 bins at a time into 128x128 matrices (W2)
  stage 3: per output block o, inverse real-DFT (256x256, matrix B3)

Between stages the spectrum moves between batch-major and spectrum-major
partition layouts (the FFT butterfly).  Partition-crossing movement uses
the DMA xbar transpose, which requires 2-byte dtypes -- so x and the
spectrum intermediates travel as fp16 (5e-4 rounding; final accumulation
is always fp32 in PSUM, output y is fp32).

Per core / per batch-chunk of 256:
  s1  (x stationary):  out[b, slot]  = x[s, b].T @ T1[s, slot]
  xbar transpose:      (b, (g,i,u)) -> ((i,u), b)  per 4-bin group g
  s2  (X stationary):  out[b, (o,u')] = Xg[(i,u), b].T @ W2[(i,u), (o,u')]
  xbar transpose:      (b, (o,kch,slot)) -> (slot, b)
  s3  (B3 stationary): out[t, b]     = B3[slot, t].T @ Y[slot, b]
"""

import numpy as np

import concourse.bacc as bacc
import concourse.bass as bass
import concourse.mybir as mybir
import concourse.tile as tile
from concourse.bass_utils import run_bass_kernel_spmd

# ---------------------------------------------------------------- constants
BATCH = 8192
FEAT = 4096
BLOCK = 256
NBLK = 16  # blocks per row (in and out)
NSLOT = 256  # real spectrum slots per block
NGRP = 32  # stage-2 groups (8 slots each)
N_CORES = 8
BC = BATCH // N_CORES  # batch rows per core (1024)
NB = 256  # batch chunk
NCHUNK = BC // NB  # 4

F32 = mybir.dt.float32
F16 = mybir.dt.float16


# ------------------------------------------------------------- host matrices
def _slot_map():
    """slot s* in 0..255 per block -> (comp, bin); comp 0 = cos, 1 = sin."""
    m = [(0, 0), (0, 128)]
    for k in range(1, 128):
        m.append((0, k))
        m.append((1, k))
    return m


_SLOTS = _slot_map()


def build_t1():
    """(256 s, 256 slots) forward real-DFT, matching np.fft.rfft."""
    s = np.arange(BLOCK)
    T1 = np.zeros((BLOCK, NSLOT), dtype=np.float64)
    for j, (comp, k) in enumerate(_SLOTS):
        ang = 2.0 * np.pi * k * s / BLOCK
        T1[:, j] = np.cos(ang) if comp == 0 else -np.sin(ang)
    return T1


def build_b3():
    """(256 slots, 256 t) inverse real-DFT, matching np.fft.irfft."""
    t = np.arange(BLOCK)
    B3 = np.zeros((NSLOT, BLOCK), dtype=np.float64)
    for j, (comp, k) in enumerate(_SLOTS):
        w = 1.0 if k in (0, 128) else 2.0
        ang = 2.0 * np.pi * k * t / BLOCK
        B3[j] = (w * np.cos(ang) if comp == 0 else -w * np.sin(ang)) / BLOCK
    return B3


def build_w2(W_real, W_imag):
    """(32, 128, 128) per-group mixing matrices.

    Group g covers slots 8g..8g+7.  Row 8i+u reads Xhat[i, slot 8g+u];
    col 8o+u' writes Yhat[o, slot 8g+u'].
    """
    Wr = np.asarray(W_real, dtype=np.float64)
    Wi = np.asarray(W_imag, dtype=np.float64)
    W2 = np.zeros((NGRP, 128, 128), dtype=np.float64)
    for g in range(NGRP):
        for u in range(8):
            comp_u, k_u = _SLOTS[8 * g + u]
            for up in range(8):
                comp_up, k_up = _SLOTS[8 * g + up]
                if k_u != k_up:
                    continue
                k = k_u
                if comp_u == 0 and comp_up == 0:
                    coef = Wr[:, :, k]  # (o, i)
                elif comp_u == 1 and comp_up == 0:
                    coef = -Wi[:, :, k]
                elif comp_u == 0 and comp_up == 1:
                    coef = Wi[:, :, k]
                else:
                    coef = Wr[:, :, k]
                W2[g, u::8, up::8] = coef.T  # [i, o]
    return W2


def pack_t1(T1):
    """(128, 2, 256) fp16: [p, kc, slot] = T1[kc*128+p, slot]."""
    return np.ascontiguousarray(T1.reshape(2, 128, NSLOT).transpose(1, 0, 2)).astype(
        np.float16
    )


def pack_b3(B3):
    """(128, 2, 256) fp16: [p, kch, t] = B3[kch*128+p, t]."""
    return np.ascontiguousarray(B3.reshape(2, 128, BLOCK).transpose(1, 0, 2)).astype(
        np.float16
    )


def pack_w2(W2):
    """(128, 32, 128) fp16: [r, g, c] = W2[g, r, c]."""
    return np.ascontiguousarray(W2.transpose(1, 0, 2)).astype(np.float16)


# ------------------------------------------------------------- device kernel
def emit_kernel(tc, outs, ins, n_chunks=NCHUNK):
    """ins: xp (NCHUNK,128,32,NB) f16 chunk-packed so each chunk load is
    128 x 16 KiB contiguous descriptors, t1 (128,2,256) f16,
    w2 (128,32,128) f16, b3 (128,2,256) f16; outs: yp like xp."""
    nc = tc.nc
    xp = ins["xp"]
    yp = outs["yp"]

    with (
        tc.tile_pool(name="wpool", bufs=1) as wpool,
        tc.tile_pool(name="xpool", bufs=2) as xpool,
        tc.tile_pool(name="spec", bufs=3) as spec,
        tc.tile_pool(name="yst", bufs=2) as ystp,
        tc.tile_pool(name="psum", bufs=8, space="PSUM") as pspool,
    ):
        t1_sb = wpool.tile([128, 2, NSLOT], F16)
        w2_sb = wpool.tile([128, NGRP, 128], F16)
        b3_sb = wpool.tile([128, 2, BLOCK], F16)
        nc.sync.dma_start(out=t1_sb, in_=ins["t1"])
        nc.sync.dma_start(out=w2_sb, in_=ins["w2"])
        nc.sync.dma_start(out=b3_sb, in_=ins["b3"])

        ncopy = 0  # alternate evacuation copies between ACT and DVE

        def evac(dst, src):
            nonlocal ncopy
            if ncopy % 2 == 0:
                nc.scalar.copy(out=dst, in_=src)
            else:
                nc.vector.tensor_copy(out=dst, in_=src)
            ncopy += 1

        xg_t = {}
        yom_t = {}

        def stage1(n):
            """x load + per-block forward DFT + shuffle 1."""
            x_sb = xpool.tile([128, 32, NB], F16, tag="x", name=f"x_{n}")
            nc.gpsimd.dma_start(out=x_sb, in_=xp[n])

            # xbt[b, bsub, g, i, u] = Xhat[b', i, slot 8g+u]
            xbt = spec.tile([128, 2, NGRP, NBLK, 8], F16, tag="xbt", name=f"xbt_{n}")
            for i in range(0, NBLK, 2):
                for bsub in range(2):
                    ps = pspool.tile([128, 2, NSLOT], F32, tag="ps", name=f"ps1_{n}")
                    for ip in range(2):
                        for kc in range(2):
                            nc.tensor.matmul(
                                ps[:, ip, :],
                                x_sb[
                                    :, 2 * (i + ip) + kc, bsub * 128 : bsub * 128 + 128
                                ],
                                t1_sb[:, kc, :],
                                start=(kc == 0),
                                stop=(kc == 1),
                            )
                    evac(
                        xbt[:, bsub, :, i : i + 2, :],
                        ps.rearrange("p i (g u) -> p g i u", u=8),
                    )

            # shuffle 1: batched xbar transpose to spectrum-major
            # xg[(i,u), bsub, g, b] = Xhat[b, i, slot 8g+u]
            xg = spec.tile([128, 2, NGRP, 128], F16, tag="xg", name=f"xg_{n}")
            nc.sync.dma_start_transpose(
                out=xg.rearrange("p s g b -> p (s g) b"), in_=xbt
            )
            xg_t[n] = xg

        def stage2(n):
            """per-bin spectral mix + shuffle 2."""
            xg = xg_t.pop(n)
            # ybt[b, bsub, o, kch, g', u'] = Yhat[b', o, slot kch*128+8g'+u']
            ybt = spec.tile(
                [128, 2, NBLK, 2, 16, 8], F16, tag="xbt", name=f"ybt_{n}"
            )
            for g in range(0, NGRP, 4):
                kch, gp = divmod(g, 16)
                for bsub in range(2):
                    ps = pspool.tile([128, 4, 128], F32, tag="ps", name=f"ps2_{n}")
                    for q in range(4):
                        nc.tensor.matmul(
                            ps[:, q, :],
                            xg[:, bsub, g + q, :],
                            w2_sb[:, g + q, :],
                            start=True,
                            stop=True,
                        )
                    evac(
                        ybt[:, bsub, :, kch, gp : gp + 4, :],
                        ps.rearrange("p q (o u) -> p o q u", u=8),
                    )

            # shuffle 2: batched xbar transpose to slot-major per block,
            # on the ACT HWDGE ring so it runs concurrently with the
            # SP-ring shuffle-1 transposes.
            # yom[p4, bsub, o, kch, b] = Yhat[b, o, slot kch*128+p4]
            yom = spec.tile([128, 2, NBLK, 2, 128], F16, tag="xg", name=f"yom_{n}")
            nc.scalar.dma_start_transpose(
                out=yom.rearrange("p s o k b -> p (s o k) b"), in_=ybt
            )
            yom_t[n] = yom

        def stage3(n):
            """per-block inverse DFT + store (fp16)."""
            yom = yom_t.pop(n)
            ybig = ystp.tile([128, 32, NB], F16, tag="ybig", name=f"ybig_{n}")
            for ob in range(0, NBLK, 4):
                pss = [
                    pspool.tile([128, 2, NB], F32, tag="ps", name=f"ps3_{n}_{ob}_{j}")
                    for j in range(4)
                ]
                for mch in range(2):
                    for kch in range(2):
                        for j in range(4):
                            nc.tensor.matmul(
                                pss[j][:, mch, :],
                                b3_sb[:, kch, mch * 128 : mch * 128 + 128],
                                yom[:, :, ob + j, kch, :],
                                start=(kch == 0),
                                stop=(kch == 1),
                            )
                for j in range(4):
                    evac(ybig[:, 2 * (ob + j) : 2 * (ob + j) + 2, :], pss[j])
            nc.gpsimd.dma_start(out=yp[n], in_=ybig)

        # software-pipelined emission: PE keeps independent work in flight
        # while each chunk's shuffles and evacuations complete.
        for k in range(n_chunks + 2):
            if 0 <= k - 2 < n_chunks:
                stage3(k - 2)
            if 0 <= k - 1 < n_chunks:
                stage2(k - 1)
            if k < n_chunks:
                stage1(k)


# ------------------------------------------------------------ host interface
_CACHED = {}


def make_inputs(W_real, W_imag):
    return {
        "t1": pack_t1(build_t1()),
        "w2": pack_w2(build_w2(W_real, W_imag)),
        "b3": pack_b3(build_b3()),
    }


def _build_bass():
    if "nc" in _CACHED:
        return _CACHED["nc"]
    nc = bacc.Bacc("TRN2", target_bir_lowering=False, debug=False)
    ins = {
        "xp": nc.dram_tensor(
            "xp", [NCHUNK, 128, 32, NB], F16, kind="ExternalInput"
        ).ap(),
        "t1": nc.dram_tensor("t1", [128, 2, NSLOT], F16, kind="ExternalInput").ap(),
        "w2": nc.dram_tensor("w2", [128, NGRP, 128], F16, kind="ExternalInput").ap(),
        "b3": nc.dram_tensor("b3", [128, 2, BLOCK], F16, kind="ExternalInput").ap(),
    }
    outs = {
        "yp": nc.dram_tensor(
            "yp", [NCHUNK, 128, 32, NB], F16, kind="ExternalOutput"
        ).ap()
    }
    with tile.TileContext(nc) as tc:
        emit_kernel(tc, outs, ins, NCHUNK)
    nc.compile()
    _CACHED["nc"] = nc
    return nc


def run_sharded(x, W_real, W_imag, trace=False):
    """Run on 8 NeuronCores; returns (y, BassKernelResults)."""
    x = np.asarray(x, dtype=np.float32)
    w = make_inputs(W_real, W_imag)

    in_maps = []
    for c in range(N_CORES):
        xc = x[c * BC : (c + 1) * BC, :]
        # xp[n, p, r, b] = x[n*NB+b, r*128+p]
        xp = np.ascontiguousarray(
            xc.reshape(NCHUNK, NB, 32, 128).transpose(0, 3, 2, 1)
        ).astype(np.float16)
        in_maps.append({"xp": xp, **w})

    nc = _build_bass()
    res = run_bass_kernel_spmd(nc, in_maps, core_ids=list(range(N_CORES)), trace=trace)

    y = np.empty((BATCH, FEAT), dtype=np.float32)
    for c in range(N_CORES):
        ypc = res.results[c]["yp"]  # [NCHUNK, 128, 32, NB] f16
        y[c * BC : (c + 1) * BC, :] = (
            ypc.transpose(0, 3, 2, 1).reshape(BC, FEAT).astype(np.float32)
        )
    return y, res


def kernel(x, W_real, W_imag):
    y, _ = run_sharded(x, W_real, W_imag, trace=False)
    return y



# revision 26
# speedup vs baseline: 1.6919x; 1.1214x over previous
"""Trainium2 Bass kernel for nn_BlockFourierCirculant.

Math: y = irfft( einsum('oik,bik->bok', Wf, rfft(x.reshape(b,16,256))) )
with 4096 features = 16 blocks x 256 and a 129-bin half-spectrum.

The op is linear per batch row and factorizes into three matmul stages
(data-parallel over batch across 8 cores):

  stage 1: per input block i, forward real-DFT (256x256, matrix T1)
  stage 2: per frequency bin, a 32x32 real "complex multiply + block mix",
           bundled 4 bins at a time into 128x128 matrices (W2)
  stage 3: per output block o, inverse real-DFT (256x256, matrix B3)

Between stages the spectrum moves between batch-major and spectrum-major
partition layouts (the FFT butterfly).  Partition-crossing movement uses
the DMA xbar transpose, which requires 2-byte dtypes -- so x and the
spectrum intermediates travel as fp16 (5e-4 rounding; final accumulation
is always fp32 in PSUM, output y is fp32).

Per core / per batch-chunk of 256:
  s1  (x stationary):  out[b, slot]  = x[s, b].T @ T1[s, slot]
  xbar transpose:      (b, (g,i,u)) -> ((i,u), b)  per 4-bin group g
  s2  (X stationary):  out[b, (o,u')] = Xg[(i,u), b].T @ W2[(i,u), (o,u')]
  xbar transpose:      (b, (o,kch,slot)) -> (slot, b)
  s3  (B3 stationary): out[t, b]     = B3[slot, t].T @ Y[slot, b]
"""

import numpy as np

import concourse.bacc as bacc
import concourse.bass as bass
import concourse.mybir as mybir
import concourse.tile as tile
from concourse.bass_utils import run_bass_kernel_spmd

# ---------------------------------------------------------------- constants
BATCH = 8192
FEAT = 4096
BLOCK = 256
NBLK = 16  # blocks per row (in and out)
NSLOT = 256  # real spectrum slots per block
NGRP = 32  # stage-2 groups (8 slots each)
N_CORES = 8
BC = BATCH // N_CORES  # batch rows per core (1024)
NB = 256  # batch chunk
NCHUNK = BC // NB  # 4

F32 = mybir.dt.float32
F16 = mybir.dt.float16


# ------------------------------------------------------------- host matrices
def _slot_map():
    """slot s* in 0..255 per block -> (comp, bin); comp 0 = cos, 1 = sin."""
    m = [(0, 0), (0, 128)]
    for k in range(1, 128):
        m.append((0, k))
        m.append((1, k))
    return m


_SLOTS = _slot_map()


def build_t1():
    """(256 s, 256 slots) forward real-DFT, matching np.fft.rfft."""
    s = np.arange(BLOCK)
    T1 = np.zeros((BLOCK, NSLOT), dtype=np.float64)
    for j, (comp, k) in enumerate(_SLOTS):
        ang = 2.0 * np.pi * k * s / BLOCK
        T1[:, j] = np.cos(ang) if comp == 0 else -np.sin(ang)
    return T1


def build_b3():
    """(256 slots, 256 t) inverse real-DFT, matching np.fft.irfft."""
    t = np.arange(BLOCK)
    B3 = np.zeros((NSLOT, BLOCK), dtype=np.float64)
    for j, (comp, k) in enumerate(_SLOTS):
        w = 1.0 if k in (0, 128) else 2.0
        ang = 2.0 * np.pi * k * t / BLOCK
        B3[j] = (w * np.cos(ang) if comp == 0 else -w * np.sin(ang)) / BLOCK
    return B3


def build_w2(W_real, W_imag):
    """(32, 128, 128) per-group mixing matrices.

    Group g covers slots 8g..8g+7.  Row 8i+u reads Xhat[i, slot 8g+u];
    col 8o+u' writes Yhat[o, slot 8g+u'].
    """
    Wr = np.asarray(W_real, dtype=np.float64)
    Wi = np.asarray(W_imag, dtype=np.float64)
    W2 = np.zeros((NGRP, 128, 128), dtype=np.float64)
    for g in range(NGRP):
        for u in range(8):
            comp_u, k_u = _SLOTS[8 * g + u]
            for up in range(8):
                comp_up, k_up = _SLOTS[8 * g + up]
                if k_u != k_up:
                    continue
                k = k_u
                if comp_u == 0 and comp_up == 0:
                    coef = Wr[:, :, k]  # (o, i)
                elif comp_u == 1 and comp_up == 0:
                    coef = -Wi[:, :, k]
                elif comp_u == 0 and comp_up == 1:
                    coef = Wi[:, :, k]
                else:
                    coef = Wr[:, :, k]
                W2[g, u::8, up::8] = coef.T  # [i, o]
    return W2


def pack_t1(T1):
    """(128, 2, 256) fp16: [p, kc, slot] = T1[kc*128+p, slot]."""
    return np.ascontiguousarray(T1.reshape(2, 128, NSLOT).transpose(1, 0, 2)).astype(
        np.float16
    )


def pack_b3(B3):
    """(128, 2, 256) fp16: [p, kch, t] = B3[kch*128+p, t]."""
    return np.ascontiguousarray(B3.reshape(2, 128, BLOCK).transpose(1, 0, 2)).astype(
        np.float16
    )


def pack_w2(W2):
    """(128, 32, 128) fp16: [r, g, c] = W2[g, r, c]."""
    return np.ascontiguousarray(W2.transpose(1, 0, 2)).astype(np.float16)


# ------------------------------------------------------------- device kernel
def emit_kernel(tc, outs, ins, n_chunks=NCHUNK):
    """ins: xp (NCHUNK,128,32,NB) f16 chunk-packed so each chunk load is
    128 x 16 KiB contiguous descriptors, t1 (128,2,256) f16,
    w2 (128,32,128) f16, b3 (128,2,256) f16; outs: yp like xp."""
    nc = tc.nc
    xp = ins["xp"]
    yp = outs["yp"]

    with (
        tc.tile_pool(name="wpool", bufs=1) as wpool,
        tc.tile_pool(name="xpool", bufs=2) as xpool,
        tc.tile_pool(name="spec", bufs=3) as spec,
        tc.tile_pool(name="yst", bufs=2) as ystp,
        tc.tile_pool(name="psum", bufs=8, space="PSUM") as pspool,
    ):
        t1_sb = wpool.tile([128, 2, NSLOT], F16)
        w2_sb = wpool.tile([128, NGRP, 128], F16)
        b3_sb = wpool.tile([128, 2, BLOCK], F16)
        nc.sync.dma_start(out=t1_sb, in_=ins["t1"])
        nc.sync.dma_start(out=w2_sb, in_=ins["w2"])
        nc.sync.dma_start(out=b3_sb, in_=ins["b3"])

        ncopy = 0  # alternate evacuation copies between ACT and DVE

        def evac(dst, src):
            nonlocal ncopy
            if ncopy % 2 == 0:
                nc.scalar.copy(out=dst, in_=src)
            else:
                nc.vector.tensor_copy(out=dst, in_=src)
            ncopy += 1

        xg_t = {}
        yom_t = {}

        def stage1(n):
            """x load + per-block forward DFT + shuffle 1."""
            x_sb = xpool.tile([128, 32, NB], F16, tag="x", name=f"x_{n}")
            # two half-loads so the first matmuls start sooner
            nc.gpsimd.dma_start(out=x_sb[:, :16, :], in_=xp[n, :, :16, :])
            nc.gpsimd.dma_start(out=x_sb[:, 16:, :], in_=xp[n, :, 16:, :])

            # xbt[b, bsub, g, i, u] = Xhat[b', i, slot 8g+u]
            xbt = spec.tile([128, 2, NGRP, NBLK, 8], F16, tag="xbt", name=f"xbt_{n}")
            for i in range(0, NBLK, 2):
                for bsub in range(2):
                    ps = pspool.tile([128, 2, NSLOT], F32, tag="ps", name=f"ps1_{n}")
                    for ip in range(2):
                        for kc in range(2):
                            nc.tensor.matmul(
                                ps[:, ip, :],
                                x_sb[
                                    :, 2 * (i + ip) + kc, bsub * 128 : bsub * 128 + 128
                                ],
                                t1_sb[:, kc, :],
                                start=(kc == 0),
                                stop=(kc == 1),
                            )
                    evac(
                        xbt[:, bsub, :, i : i + 2, :],
                        ps.rearrange("p i (g u) -> p g i u", u=8),
                    )

            # shuffle 1: batched xbar transpose to spectrum-major
            # xg[(i,u), bsub, g, b] = Xhat[b, i, slot 8g+u]
            xg = spec.tile(
                [128, 2, NGRP, 128], F16, tag="xg", bufs=4, name=f"xg_{n}"
            )
            nc.sync.dma_start_transpose(
                out=xg.rearrange("p s g b -> p (s g) b"), in_=xbt
            )
            xg_t[n] = xg

        def stage2(n):
            """per-bin spectral mix + shuffle 2."""
            xg = xg_t.pop(n)
            # ybt[b, bsub, o, kch, g', u'] = Yhat[b', o, slot kch*128+8g'+u']
            ybt = spec.tile(
                [128, 2, NBLK, 2, 16, 8], F16, tag="xbt", name=f"ybt_{n}"
            )
            for g in range(0, NGRP, 4):
                kch, gp = divmod(g, 16)
                for bsub in range(2):
                    ps = pspool.tile([128, 4, 128], F32, tag="ps", name=f"ps2_{n}")
                    for q in range(4):
                        nc.tensor.matmul(
                            ps[:, q, :],
                            xg[:, bsub, g + q, :],
                            w2_sb[:, g + q, :],
                            start=True,
                            stop=True,
                        )
                    evac(
                        ybt[:, bsub, :, kch, gp : gp + 4, :],
                        ps.rearrange("p q (o u) -> p o q u", u=8),
                    )

            # shuffle 2: batched xbar transpose to slot-major per block
            # yom[p4, bsub, o, kch, b] = Yhat[b, o, slot kch*128+p4]
            yom = spec.tile(
                [128, 2, NBLK, 2, 128], F16, tag="xg", bufs=4, name=f"yom_{n}"
            )
            nc.sync.dma_start_transpose(
                out=yom.rearrange("p s o k b -> p (s o k) b"), in_=ybt
            )
            yom_t[n] = yom

        def stage3(n):
            """per-block inverse DFT + store (fp16)."""
            yom = yom_t.pop(n)
            ybig = ystp.tile([128, 32, NB], F16, tag="ybig", name=f"ybig_{n}")
            for ob in range(0, NBLK, 4):
                pss = [
                    pspool.tile([128, 2, NB], F32, tag="ps", name=f"ps3_{n}_{ob}_{j}")
                    for j in range(4)
                ]
                for mch in range(2):
                    for kch in range(2):
                        for j in range(4):
                            nc.tensor.matmul(
                                pss[j][:, mch, :],
                                b3_sb[:, kch, mch * 128 : mch * 128 + 128],
                                yom[:, :, ob + j, kch, :],
                                start=(kch == 0),
                                stop=(kch == 1),
                            )
                for j in range(4):
                    evac(ybig[:, 2 * (ob + j) : 2 * (ob + j) + 2, :], pss[j])
            nc.gpsimd.dma_start(out=yp[n], in_=ybig)

        # software-pipelined emission.  stage1(k) is emitted BEFORE
        # stage2(k-1)/stage3(k-2) so the PE's instruction stream has
        # transpose-independent work queued ahead of the stages that
        # block on the shuffles (stage2/stage3); the transposes then
        # overlap s1 compute instead of stalling the whole PE stream.
        for k in range(n_chunks + 2):
            if k < n_chunks:
                stage1(k)
            if 0 <= k - 1 < n_chunks:
                stage2(k - 1)
            if 0 <= k - 2 < n_chunks:
                stage3(k - 2)


# ------------------------------------------------------------ host interface
_CACHED = {}


def make_inputs(W_real, W_imag):
    return {
        "t1": pack_t1(build_t1()),
        "w2": pack_w2(build_w2(W_real, W_imag)),
        "b3": pack_b3(build_b3()),
    }


def _build_bass():
    if "nc" in _CACHED:
        return _CACHED["nc"]
    nc = bacc.Bacc("TRN2", target_bir_lowering=False, debug=False)
    ins = {
        "xp": nc.dram_tensor(
            "xp", [NCHUNK, 128, 32, NB], F16, kind="ExternalInput"
        ).ap(),
        "t1": nc.dram_tensor("t1", [128, 2, NSLOT], F16, kind="ExternalInput").ap(),
        "w2": nc.dram_tensor("w2", [128, NGRP, 128], F16, kind="ExternalInput").ap(),
        "b3": nc.dram_tensor("b3", [128, 2, BLOCK], F16, kind="ExternalInput").ap(),
    }
    outs = {
        "yp": nc.dram_tensor(
            "yp", [NCHUNK, 128, 32, NB], F16, kind="ExternalOutput"
        ).ap()
    }
    with tile.TileContext(nc) as tc:
        emit_kernel(tc, outs, ins, NCHUNK)
    nc.compile()
    _CACHED["nc"] = nc
    return nc


def run_sharded(x, W_real, W_imag, trace=False):
    """Run on 8 NeuronCores; returns (y, BassKernelResults)."""
    x = np.asarray(x, dtype=np.float32)
    w = make_inputs(W_real, W_imag)

    in_maps = []
    for c in range(N_CORES):
        xc = x[c * BC : (c + 1) * BC, :]
        # xp[n, p, r, b] = x[n*NB+b, r*128+p]
        xp = np.ascontiguousarray(
            xc.reshape(NCHUNK, NB, 32, 128).transpose(0, 3, 2, 1)
        ).astype(np.float16)
        in_maps.append({"xp": xp, **w})

    nc = _build_bass()
    res = run_bass_kernel_spmd(nc, in_maps, core_ids=list(range(N_CORES)), trace=trace)

    y = np.empty((BATCH, FEAT), dtype=np.float32)
    for c in range(N_CORES):
        ypc = res.results[c]["yp"]  # [NCHUNK, 128, 32, NB] f16
        y[c * BC : (c + 1) * BC, :] = (
            ypc.transpose(0, 3, 2, 1).reshape(BC, FEAT).astype(np.float32)
        )
    return y, res


def kernel(x, W_real, W_imag):
    y, _ = run_sharded(x, W_real, W_imag, trace=False)
    return y



# revision 32
# speedup vs baseline: 1.7656x; 1.0435x over previous
"""Trainium2 Bass kernel for nn_BlockFourierCirculant.

Math: y = irfft( einsum('oik,bik->bok', Wf, rfft(x.reshape(b,16,256))) )
with 4096 features = 16 blocks x 256 and a 129-bin half-spectrum.

The op is linear per batch row and factorizes into three matmul stages
(data-parallel over batch across 8 cores):

  stage 1: per input block i, forward real-DFT (256x256, matrix T1)
  stage 2: per frequency bin, a 32x32 real "complex multiply + block mix",
           bundled 4 bins at a time into 128x128 matrices (W2)
  stage 3: per output block o, inverse real-DFT (256x256, matrix B3)

Between stages the spectrum moves between batch-major and spectrum-major
partition layouts (the FFT butterfly).  Partition-crossing movement uses
the DMA xbar transpose, which requires 2-byte dtypes -- so x and the
spectrum intermediates travel as fp16 (5e-4 rounding; final accumulation
is always fp32 in PSUM, output y is fp32).

Per core / per batch-chunk of 256:
  s1  (x stationary):  out[b, slot]  = x[s, b].T @ T1[s, slot]
  xbar transpose:      (b, (g,i,u)) -> ((i,u), b)  per 4-bin group g
  s2  (X stationary):  out[b, (o,u')] = Xg[(i,u), b].T @ W2[(i,u), (o,u')]
  xbar transpose:      (b, (o,kch,slot)) -> (slot, b)
  s3  (B3 stationary): out[t, b]     = B3[slot, t].T @ Y[slot, b]
"""

import numpy as np

import concourse.bacc as bacc
import concourse.bass as bass
import concourse.mybir as mybir
import concourse.tile as tile
from concourse.bass_utils import run_bass_kernel_spmd

# ---------------------------------------------------------------- constants
BATCH = 8192
FEAT = 4096
BLOCK = 256
NBLK = 16  # blocks per row (in and out)
NSLOT = 256  # real spectrum slots per block
NGRP = 32  # stage-2 groups (8 slots each)
N_CORES = 8
BC = BATCH // N_CORES  # batch rows per core (1024)
NB = 256  # batch chunk
NCHUNK = BC // NB  # 4

F32 = mybir.dt.float32
F16 = mybir.dt.float16


# ------------------------------------------------------------- host matrices
def _slot_map():
    """slot s* in 0..255 per block -> (comp, bin); comp 0 = cos, 1 = sin."""
    m = [(0, 0), (0, 128)]
    for k in range(1, 128):
        m.append((0, k))
        m.append((1, k))
    return m


_SLOTS = _slot_map()


def build_t1():
    """(256 s, 256 slots) forward real-DFT, matching np.fft.rfft."""
    s = np.arange(BLOCK)
    T1 = np.zeros((BLOCK, NSLOT), dtype=np.float64)
    for j, (comp, k) in enumerate(_SLOTS):
        ang = 2.0 * np.pi * k * s / BLOCK
        T1[:, j] = np.cos(ang) if comp == 0 else -np.sin(ang)
    return T1


def build_b3():
    """(256 slots, 256 t) inverse real-DFT, matching np.fft.irfft."""
    t = np.arange(BLOCK)
    B3 = np.zeros((NSLOT, BLOCK), dtype=np.float64)
    for j, (comp, k) in enumerate(_SLOTS):
        w = 1.0 if k in (0, 128) else 2.0
        ang = 2.0 * np.pi * k * t / BLOCK
        B3[j] = (w * np.cos(ang) if comp == 0 else -w * np.sin(ang)) / BLOCK
    return B3


def build_w2(W_real, W_imag):
    """(32, 128, 128) per-group mixing matrices.

    Group g covers slots 8g..8g+7.  Row 8i+u reads Xhat[i, slot 8g+u];
    col 8o+u' writes Yhat[o, slot 8g+u'].
    """
    Wr = np.asarray(W_real, dtype=np.float64)
    Wi = np.asarray(W_imag, dtype=np.float64)
    W2 = np.zeros((NGRP, 128, 128), dtype=np.float64)
    for g in range(NGRP):
        for u in range(8):
            comp_u, k_u = _SLOTS[8 * g + u]
            for up in range(8):
                comp_up, k_up = _SLOTS[8 * g + up]
                if k_u != k_up:
                    continue
                k = k_u
                if comp_u == 0 and comp_up == 0:
                    coef = Wr[:, :, k]  # (o, i)
                elif comp_u == 1 and comp_up == 0:
                    coef = -Wi[:, :, k]
                elif comp_u == 0 and comp_up == 1:
                    coef = Wi[:, :, k]
                else:
                    coef = Wr[:, :, k]
                W2[g, u::8, up::8] = coef.T  # [i, o]
    return W2


def pack_t1(T1):
    """(128, 2, 256) fp16: [p, kc, slot] = T1[kc*128+p, slot]."""
    return np.ascontiguousarray(T1.reshape(2, 128, NSLOT).transpose(1, 0, 2)).astype(
        np.float16
    )


def pack_b3(B3):
    """(128, 2, 256) fp16: [p, kch, t] = B3[kch*128+p, t]."""
    return np.ascontiguousarray(B3.reshape(2, 128, BLOCK).transpose(1, 0, 2)).astype(
        np.float16
    )


def pack_w2(W2):
    """(128, 32, 128) fp16: [r, g, c] = W2[g, r, c]."""
    return np.ascontiguousarray(W2.transpose(1, 0, 2)).astype(np.float16)


# ------------------------------------------------------------- device kernel
def emit_kernel(tc, outs, ins, n_chunks=NCHUNK):
    """ins: xp (NCHUNK,128,32,NB) f16 chunk-packed so each chunk load is
    128 x 16 KiB contiguous descriptors, t1 (128,2,256) f16,
    w2 (128,32,128) f16, b3 (128,2,256) f16; outs: yp like xp."""
    nc = tc.nc
    xp = ins["xp"]
    yp = outs["yp"]

    with (
        tc.tile_pool(name="wpool", bufs=1) as wpool,
        tc.tile_pool(name="xpool", bufs=4) as xpool,
        tc.tile_pool(name="spec", bufs=3) as spec,
        tc.tile_pool(name="yst", bufs=2) as ystp,
        tc.tile_pool(name="psum", bufs=8, space="PSUM") as pspool,
    ):
        t1_sb = wpool.tile([128, 2, NSLOT], F16)
        w2_sb = wpool.tile([128, NGRP, 128], F16)
        b3_sb = wpool.tile([128, 2, BLOCK], F16)
        nc.sync.dma_start(out=t1_sb, in_=ins["t1"])
        nc.sync.dma_start(out=w2_sb, in_=ins["w2"])
        nc.sync.dma_start(out=b3_sb, in_=ins["b3"])

        ncopy = 0  # alternate evacuation copies between ACT and DVE

        def evac(dst, src):
            nonlocal ncopy
            if ncopy % 2 == 0:
                nc.scalar.copy(out=dst, in_=src)
            else:
                nc.vector.tensor_copy(out=dst, in_=src)
            ncopy += 1

        xg_t = {}
        yom_t = {}

        def stage1(n):
            """x load + per-block forward DFT + shuffle 1."""
            # two half-chunk tiles so the first matmuls start sooner and
            # the pool rotates at half-chunk granularity
            xh = []
            for hf in range(2):
                t = xpool.tile([128, 16, NB], F16, tag="x", name=f"x_{n}_{hf}")
                nc.gpsimd.dma_start(out=t, in_=xp[n, :, 16 * hf : 16 * hf + 16, :])
                xh.append(t)

            def x_slab(r):
                return xh[r // 16][:, r % 16, :]

            # xbt[b, bsub, g, i, u] = Xhat[b', i, slot 8g+u]
            xbt = spec.tile([128, 2, NGRP, NBLK, 8], F16, tag="xbt", name=f"xbt_{n}")
            for i in range(0, NBLK, 2):
                for bsub in range(2):
                    ps = pspool.tile([128, 2, NSLOT], F32, tag="ps", name=f"ps1_{n}")
                    for ip in range(2):
                        for kc in range(2):
                            nc.tensor.matmul(
                                ps[:, ip, :],
                                x_slab(2 * (i + ip) + kc)[
                                    :, bsub * 128 : bsub * 128 + 128
                                ],
                                t1_sb[:, kc, :],
                                start=(kc == 0),
                                stop=(kc == 1),
                            )
                    evac(
                        xbt[:, bsub, :, i : i + 2, :],
                        ps.rearrange("p i (g u) -> p g i u", u=8),
                    )

            # shuffle 1: batched xbar transpose to spectrum-major
            # xg[(i,u), bsub, g, b] = Xhat[b, i, slot 8g+u]
            xg = spec.tile(
                [128, 2, NGRP, 128], F16, tag="xg", bufs=3, name=f"xg_{n}"
            )
            nc.sync.dma_start_transpose(
                out=xg.rearrange("p s g b -> p (s g) b"), in_=xbt
            )
            xg_t[n] = xg

        def stage2(n):
            """per-bin spectral mix + shuffle 2."""
            xg = xg_t.pop(n)
            # ybt[b, bsub, o, kch, g', u'] = Yhat[b', o, slot kch*128+8g'+u']
            ybt = spec.tile(
                [128, 2, NBLK, 2, 16, 8], F16, tag="xbt", name=f"ybt_{n}"
            )
            for g in range(0, NGRP, 4):
                kch, gp = divmod(g, 16)
                for bsub in range(2):
                    ps = pspool.tile([128, 4, 128], F32, tag="ps", name=f"ps2_{n}")
                    for q in range(4):
                        nc.tensor.matmul(
                            ps[:, q, :],
                            xg[:, bsub, g + q, :],
                            w2_sb[:, g + q, :],
                            start=True,
                            stop=True,
                        )
                    evac(
                        ybt[:, bsub, :, kch, gp : gp + 4, :],
                        ps.rearrange("p q (o u) -> p o q u", u=8),
                    )

            # shuffle 2: batched xbar transpose to slot-major per block
            # yom[p4, bsub, o, kch, b] = Yhat[b, o, slot kch*128+p4]
            yom = spec.tile(
                [128, 2, NBLK, 2, 128], F16, tag="yom", bufs=2, name=f"yom_{n}"
            )
            nc.sync.dma_start_transpose(
                out=yom.rearrange("p s o k b -> p (s o k) b"), in_=ybt
            )
            yom_t[n] = yom

        def stage3(n):
            """per-block inverse DFT + store (fp16)."""
            yom = yom_t.pop(n)
            ybig = ystp.tile([128, 32, NB], F16, tag="ybig", name=f"ybig_{n}")
            for ob in range(0, NBLK, 4):
                pss = [
                    pspool.tile([128, 2, NB], F32, tag="ps", name=f"ps3_{n}_{ob}_{j}")
                    for j in range(4)
                ]
                for mch in range(2):
                    for kch in range(2):
                        for j in range(4):
                            nc.tensor.matmul(
                                pss[j][:, mch, :],
                                b3_sb[:, kch, mch * 128 : mch * 128 + 128],
                                yom[:, :, ob + j, kch, :],
                                start=(kch == 0),
                                stop=(kch == 1),
                            )
                for j in range(4):
                    evac(ybig[:, 2 * (ob + j) : 2 * (ob + j) + 2, :], pss[j])
            nc.gpsimd.dma_start(out=yp[n], in_=ybig)

        # software-pipelined emission with a 3-chunk stage1 lead: the PE
        # stream always has transpose-independent s1 work queued ahead of
        # the stages that block on the shuffles, so the xbar transposes
        # overlap compute instead of stalling the whole PE stream.
        lead = min(3, n_chunks)
        for k in range(lead):
            stage1(k)
        for k in range(n_chunks):
            stage2(k)
            if k + lead < n_chunks:
                stage1(k + lead)
            if k >= 1:
                stage3(k - 1)
        stage3(n_chunks - 1)


# ------------------------------------------------------------ host interface
_CACHED = {}


def make_inputs(W_real, W_imag):
    return {
        "t1": pack_t1(build_t1()),
        "w2": pack_w2(build_w2(W_real, W_imag)),
        "b3": pack_b3(build_b3()),
    }


def _build_bass():
    if "nc" in _CACHED:
        return _CACHED["nc"]
    nc = bacc.Bacc("TRN2", target_bir_lowering=False, debug=False)
    ins = {
        "xp": nc.dram_tensor(
            "xp", [NCHUNK, 128, 32, NB], F16, kind="ExternalInput"
        ).ap(),
        "t1": nc.dram_tensor("t1", [128, 2, NSLOT], F16, kind="ExternalInput").ap(),
        "w2": nc.dram_tensor("w2", [128, NGRP, 128], F16, kind="ExternalInput").ap(),
        "b3": nc.dram_tensor("b3", [128, 2, BLOCK], F16, kind="ExternalInput").ap(),
    }
    outs = {
        "yp": nc.dram_tensor(
            "yp", [NCHUNK, 128, 32, NB], F16, kind="ExternalOutput"
        ).ap()
    }
    with tile.TileContext(nc) as tc:
        emit_kernel(tc, outs, ins, NCHUNK)
    nc.compile()
    _CACHED["nc"] = nc
    return nc


def run_sharded(x, W_real, W_imag, trace=False):
    """Run on 8 NeuronCores; returns (y, BassKernelResults)."""
    x = np.asarray(x, dtype=np.float32)
    w = make_inputs(W_real, W_imag)

    in_maps = []
    for c in range(N_CORES):
        xc = x[c * BC : (c + 1) * BC, :]
        # xp[n, p, r, b] = x[n*NB+b, r*128+p]
        xp = np.ascontiguousarray(
            xc.reshape(NCHUNK, NB, 32, 128).transpose(0, 3, 2, 1)
        ).astype(np.float16)
        in_maps.append({"xp": xp, **w})

    nc = _build_bass()
    res = run_bass_kernel_spmd(nc, in_maps, core_ids=list(range(N_CORES)), trace=trace)

    y = np.empty((BATCH, FEAT), dtype=np.float32)
    for c in range(N_CORES):
        ypc = res.results[c]["yp"]  # [NCHUNK, 128, 32, NB] f16
        y[c * BC : (c + 1) * BC, :] = (
            ypc.transpose(0, 3, 2, 1).reshape(BC, FEAT).astype(np.float32)
        )
    return y, res


def kernel(x, W_real, W_imag):
    y, _ = run_sharded(x, W_real, W_imag, trace=False)
    return y

